# revision 7
# baseline (speedup 1.0000x reference)
"""Bayesian linear layer (per-sample weights) on 8 Trainium2 NeuronCores.

out[b,o] = sum_i x[b,i] * (eps[b,i,o]*softplus(ro)[i,o] + mu[i,o])
           + eps_bias[b,o]*softplus(ro_bias)[o] + mu_bias[o]

Strategy (data-parallel over batch, 16 samples per core):
  - eps shard (16,1024,1024 f32, 64MB) is streamed in [128, 4096] tiles
    (i-chunk on partitions, 4 chunks x o on free dim; 2MB contiguous DMA).
  - DVE multiplies each tile by the matching softplus(ro) tile in-place.
  - TensorE reduces over i with M=1 matmuls: lhsT = x[b, chunk] column
    ([128,1]), rhs = scaled eps tile slice ([128,512]), accumulated in a
    [1,1024] PSUM tile per sample.
  - The x@mu term is computed once per core with M=16 matmuls and folded
    (together with the bias terms) into a per-sample bias row that is added
    to the PSUM result before store.
"""

import numpy as np

import concourse.bass as bass
import concourse.bacc as bacc
import concourse.mybir as mybir
from concourse.tile import TileContext
from concourse.bass_utils import run_bass_kernel_spmd

F32 = mybir.dt.float32
AF = mybir.ActivationFunctionType

B, IN, OUT = 128, 1024, 1024
NCORES = 8
BS = B // NCORES          # 16 samples per core
P = 128                   # partitions
NCH = IN // P             # 8 i-chunks
HALF_CH = NCH // 2        # 4 chunks per eps tile
HALF_F = HALF_CH * OUT    # 4096 free elems per eps tile
NH = OUT // 512           # 2 matmul halves (PSUM bank = 512 f32)


def build_nc():
    nc = bacc.Bacc(None, target_bir_lowering=False)

    eps_d = nc.declare_dram_parameter("eps", [BS, IN, OUT], F32, isOutput=False)
    ro_d = nc.declare_dram_parameter("ro", [IN, OUT], F32, isOutput=False)
    mu_d = nc.declare_dram_parameter("mu", [IN, OUT], F32, isOutput=False)
    # xt[p, c*BS + b] = x[b, c*128 + p]  (host-side layout transform)
    xt_d = nc.declare_dram_parameter("xt", [P, NCH * BS], F32, isOutput=False)
    eb_d = nc.declare_dram_parameter("eps_bias", [BS, OUT], F32, isOutput=False)
    rb_d = nc.declare_dram_parameter("ro_bias", [1, OUT], F32, isOutput=False)
    mb_d = nc.declare_dram_parameter("mu_bias", [1, OUT], F32, isOutput=False)
    out_d = nc.declare_dram_parameter("out", [BS, OUT], F32, isOutput=True)

    with TileContext(nc) as tc:
        with (
            tc.tile_pool(name="const", bufs=1) as cpool,
            tc.tile_pool(name="eps", bufs=3) as epool,
            tc.tile_pool(name="small", bufs=2) as spool,
            tc.tile_pool(name="psmu", bufs=1, space="PSUM") as pmupool,
            tc.tile_pool(name="psum", bufs=3, space="PSUM") as ppool,
        ):
            # ---- resident constants -------------------------------------
            sig = cpool.tile([P, NCH * OUT], F32)     # softplus(ro), chunked
            nc.sync.dma_start(out=sig, in_=ro_d.rearrange("(c p) o -> p c o", p=P))
            nc.scalar.activation(sig, sig, AF.Exp)
            nc.scalar.activation(sig, sig, AF.Ln, bias=1.0)

            mu_sb = cpool.tile([P, NCH * OUT], F32)
            nc.sync.dma_start(out=mu_sb, in_=mu_d.rearrange("(c p) o -> p c o", p=P))

            xt = cpool.tile([P, NCH * BS], F32)
            nc.scalar.dma_start(out=xt, in_=xt_d[:, :])

            # ---- per-sample bias row (includes x@mu term) ---------------
            eb16 = cpool.tile([BS, OUT], F32)
            nc.scalar.dma_start(out=eb16, in_=eb_d[:, :])
            rb16 = cpool.tile([BS, OUT], F32)
            mb16 = cpool.tile([BS, OUT], F32)
            for b in range(BS):
                nc.scalar.dma_start(out=rb16[b : b + 1, :], in_=rb_d[:, :])
                nc.scalar.dma_start(out=mb16[b : b + 1, :], in_=mb_d[:, :])
            nc.scalar.activation(rb16, rb16, AF.Exp)
            nc.scalar.activation(rb16, rb16, AF.Ln, bias=1.0)

            bias16 = cpool.tile([BS, OUT], F32)
            nc.vector.tensor_mul(out=bias16, in0=eb16, in1=rb16)
            nc.vector.tensor_add(out=bias16, in0=bias16, in1=mb16)

            # x @ mu via M=16 matmuls, accumulated over the 8 i-chunks
            psmu = pmupool.tile([BS, OUT], F32)
            for c in range(NCH):
                for nh in range(NH):
                    nc.tensor.matmul(
                        psmu[:, nh * 512 : (nh + 1) * 512],
                        xt[:, c * BS : (c + 1) * BS],
                        mu_sb[:, c * OUT + nh * 512 : c * OUT + (nh + 1) * 512],
                        start=(c == 0),
                        stop=(c == NCH - 1),
                    )
            nc.vector.tensor_add(out=bias16, in0=bias16, in1=psmu)

            # flatten [16, 1024] -> [1, 16384] so it lives on partition 0
            bias_row = cpool.tile([1, BS * OUT], F32)
            nc.scalar.dma_start(out=bias_row, in_=bias16)

            # ---- main streaming loop ------------------------------------
            for b in range(BS):
                ps = ppool.tile([1, OUT], F32)
                for h in range(2):
                    ep = epool.tile([P, HALF_F], F32)
                    src = eps_d[b, h * 512 : (h + 1) * 512, :]
                    nc.sync.dma_start(
                        out=ep, in_=src.rearrange("(c p) o -> p c o", p=P)
                    )
                    nc.vector.tensor_mul(
                        out=ep, in0=ep,
                        in1=sig[:, h * HALF_F : (h + 1) * HALF_F],
                    )
                    for c4 in range(HALF_CH):
                        c = HALF_CH * h + c4
                        col = xt[:, c * BS + b : c * BS + b + 1]
                        for nh in range(NH):
                            nc.tensor.matmul(
                                ps[0:1, nh * 512 : (nh + 1) * 512],
                                col,
                                ep[:, c4 * OUT + nh * 512 : c4 * OUT + (nh + 1) * 512],
                                start=(h == 0 and c4 == 0),
                                stop=(h == 1 and c4 == HALF_CH - 1),
                            )
                orow = spool.tile([1, OUT], F32)
                nc.vector.tensor_add(
                    out=orow, in0=ps,
                    in1=bias_row[0:1, b * OUT : (b + 1) * OUT],
                )
                nc.scalar.dma_start(out=out_d[b : b + 1, :], in_=orow)

    nc.finalize()
    return nc


_NC_CACHE = None


def _get_nc():
    global _NC_CACHE
    if _NC_CACHE is None:
        _NC_CACHE = build_nc()
    return _NC_CACHE


def kernel(x, mu, ro, mu_bias, ro_bias, eps, eps_bias, _trace=False, _tmpdir=None):
    x = np.ascontiguousarray(np.asarray(x, dtype=np.float32))
    mu = np.ascontiguousarray(np.asarray(mu, dtype=np.float32))
    ro = np.ascontiguousarray(np.asarray(ro, dtype=np.float32))
    mu_bias = np.ascontiguousarray(np.asarray(mu_bias, dtype=np.float32)).reshape(1, OUT)
    ro_bias = np.ascontiguousarray(np.asarray(ro_bias, dtype=np.float32)).reshape(1, OUT)
    eps = np.asarray(eps, dtype=np.float32)
    eps_bias = np.ascontiguousarray(np.asarray(eps_bias, dtype=np.float32))

    nc = _get_nc()

    in_maps = []
    for core in range(NCORES):
        b0, b1 = core * BS, (core + 1) * BS
        x_sh = x[b0:b1]  # (BS, IN)
        # xt[p, c*BS + b] = x_sh[b, c*128 + p]
        xt = np.ascontiguousarray(
            x_sh.reshape(BS, NCH, P).transpose(2, 1, 0).reshape(P, NCH * BS)
        )
        in_maps.append(
            {
                "eps": eps[b0:b1],
                "ro": ro,
                "mu": mu,
                "xt": xt,
                "eps_bias": eps_bias[b0:b1],
                "ro_bias": ro_bias,
                "mu_bias": mu_bias,
            }
        )

    res = run_bass_kernel_spmd(
        nc, in_maps, core_ids=list(range(NCORES)), trace=_trace, tmpdir=_tmpdir
    )
    out = np.concatenate([res.results[c]["out"] for c in range(NCORES)], axis=0)
    if _trace:
        kernel.last_results = res
    return out


# revision 11
# speedup vs baseline: 1.2687x; 1.2687x over previous
"""Bayesian linear layer (per-sample weights) on 8 Trainium2 NeuronCores.

out[b,o] = sum_i x[b,i] * (eps[b,i,o]*softplus(ro)[i,o] + mu[i,o])
           + eps_bias[b,o]*softplus(ro_bias)[o] + mu_bias[o]

Strategy (data-parallel over batch, 16 samples per core):
  - eps shard (16,1024,1024 f32, 64MB) is streamed in [128, 4096] tiles
    (i-chunk on partitions, 4 chunks x o on free dim; 2MB contiguous DMA).
  - DVE multiplies each tile by the matching softplus(ro) tile, rounding
    to float32r so TensorE can consume it at full (1 cycle/row) rate.
  - TensorE reduces over i with M=1 matmuls: lhsT = x[b, chunk] column
    ([128,1] f32r), rhs = scaled eps tile slice ([128,512] f32r),
    accumulated in a [1,1024] PSUM tile per sample.
  - The x@mu term is computed once per core with M=16 fp32 matmuls and
    folded (with the bias terms) into a per-sample bias row, bounced via
    DRAM so each sample's row is read back on partition 0 and added to
    the PSUM result before store.
"""

import numpy as np

import concourse.bass as bass
import concourse.bacc as bacc
import concourse.mybir as mybir
from concourse.tile import TileContext
from concourse.bass_utils import run_bass_kernel_spmd

F32 = mybir.dt.float32
F32R = mybir.dt.float32r
AF = mybir.ActivationFunctionType

B, IN, OUT = 128, 1024, 1024
NCORES = 8
BS = B // NCORES          # 16 samples per core
P = 128                   # partitions
NCH = IN // P             # 8 i-chunks
HALF_CH = NCH // 2        # 4 chunks per eps tile
HALF_F = HALF_CH * OUT    # 4096 free elems per eps tile
NH = OUT // 512           # 2 matmul halves (PSUM bank = 512 f32)


def build_nc():
    nc = bacc.Bacc(None, target_bir_lowering=False)

    eps_d = nc.declare_dram_parameter("eps", [BS, IN, OUT], F32, isOutput=False)
    ro_d = nc.declare_dram_parameter("ro", [IN, OUT], F32, isOutput=False)
    mu_d = nc.declare_dram_parameter("mu", [IN, OUT], F32, isOutput=False)
    # xt[p, c*BS + b] = x[b, c*128 + p]  (host-side layout transform)
    xt_d = nc.declare_dram_parameter("xt", [P, NCH * BS], F32, isOutput=False)
    eb_d = nc.declare_dram_parameter("eps_bias", [BS, OUT], F32, isOutput=False)
    rb_d = nc.declare_dram_parameter("ro_bias", [1, OUT], F32, isOutput=False)
    mb_d = nc.declare_dram_parameter("mu_bias", [1, OUT], F32, isOutput=False)
    out_d = nc.declare_dram_parameter("out", [BS, OUT], F32, isOutput=True)

    mu_r = mu_d.rearrange("(c p) o -> p c o", p=P)

    with TileContext(nc) as tc:
        with (
            tc.tile_pool(name="const", bufs=1) as cpool,
            tc.tile_pool(name="eps", bufs=3) as epool,
            tc.tile_pool(name="epr", bufs=2) as eprpool,
            tc.tile_pool(name="small", bufs=2) as spool,
            tc.tile_pool(name="dram", bufs=1, space="DRAM") as dpool,
            tc.tile_pool(name="psmu", bufs=1, space="PSUM") as pmupool,
            tc.tile_pool(name="psum", bufs=3, space="PSUM") as ppool,
        ):
            # ---- resident constants -------------------------------------
            sig = cpool.tile([P, NCH * OUT], F32)     # softplus(ro), chunked
            nc.sync.dma_start(out=sig, in_=ro_d.rearrange("(c p) o -> p c o", p=P))
            nc.scalar.activation(sig, sig, AF.Exp)
            nc.scalar.activation(sig, sig, AF.Ln, bias=1.0)

            xt = cpool.tile([P, NCH * BS], F32)
            nc.scalar.dma_start(out=xt, in_=xt_d[:, :])
            xtr = cpool.tile([P, NCH * BS], F32R)
            nc.vector.tensor_copy(out=xtr, in_=xt)

            # ---- x @ mu via M=16 matmuls (mu streamed, tiles recycled) --
            psmu = pmupool.tile([BS, OUT], F32)
            for h in range(2):
                mt = epool.tile([P, HALF_F], F32)
                nc.sync.dma_start(
                    out=mt, in_=mu_r[:, h * HALF_CH : (h + 1) * HALF_CH, :]
                )
                for c4 in range(HALF_CH):
                    c = HALF_CH * h + c4
                    for nh in range(NH):
                        nc.tensor.matmul(
                            psmu[:, nh * 512 : (nh + 1) * 512],
                            xt[:, c * BS : (c + 1) * BS],
                            mt[:, c4 * OUT + nh * 512 : c4 * OUT + (nh + 1) * 512],
                            start=(c == 0),
                            stop=(c == NCH - 1),
                        )

            # ---- per-sample bias row (includes x@mu term) ---------------
            eb16 = cpool.tile([BS, OUT], F32)
            nc.scalar.dma_start(out=eb16, in_=eb_d[:, :])
            rb16 = cpool.tile([BS, OUT], F32)
            mb16 = cpool.tile([BS, OUT], F32)
            for b in range(BS):
                nc.scalar.dma_start(out=rb16[b : b + 1, :], in_=rb_d[:, :])
                nc.scalar.dma_start(out=mb16[b : b + 1, :], in_=mb_d[:, :])
            nc.scalar.activation(rb16, rb16, AF.Exp)
            nc.scalar.activation(rb16, rb16, AF.Ln, bias=1.0)

            bias16 = cpool.tile([BS, OUT], F32)
            nc.vector.tensor_mul(out=bias16, in0=eb16, in1=rb16)
            nc.vector.tensor_add(out=bias16, in0=bias16, in1=mb16)
            nc.vector.tensor_add(out=bias16, in0=bias16, in1=psmu)

            # bounce via DRAM so each sample's bias row can be read back
            # on partition 0
            bias_dram = dpool.tile([BS, OUT], F32)
            nc.scalar.dma_start(out=bias_dram[:, :], in_=bias16)

            # ---- main streaming loop ------------------------------------
            for b in range(BS):
                ps = ppool.tile([1, OUT], F32)
                for h in range(2):
                    ep = epool.tile([P, HALF_F], F32)
                    src = eps_d[b, h * 512 : (h + 1) * 512, :]
                    nc.sync.dma_start(
                        out=ep, in_=src.rearrange("(c p) o -> p c o", p=P)
                    )
                    epr = eprpool.tile([P, HALF_F], F32R)
                    nc.vector.tensor_mul(
                        out=epr, in0=ep,
                        in1=sig[:, h * HALF_F : (h + 1) * HALF_F],
                    )
                    for c4 in range(HALF_CH):
                        c = HALF_CH * h + c4
                        col = xtr[:, c * BS + b : c * BS + b + 1]
                        for nh in range(NH):
                            nc.tensor.matmul(
                                ps[0:1, nh * 512 : (nh + 1) * 512],
                                col,
                                epr[:, c4 * OUT + nh * 512 : c4 * OUT + (nh + 1) * 512],
                                start=(h == 0 and c4 == 0),
                                stop=(h == 1 and c4 == HALF_CH - 1),
                            )
                brow = spool.tile([1, OUT], F32)
                nc.scalar.dma_start(out=brow, in_=bias_dram[b : b + 1, :])
                orow = spool.tile([1, OUT], F32)
                nc.vector.tensor_add(out=orow, in0=ps, in1=brow)
                nc.scalar.dma_start(out=out_d[b : b + 1, :], in_=orow)

    nc.finalize()
    return nc


_NC_CACHE = None


def _get_nc():
    global _NC_CACHE
    if _NC_CACHE is None:
        _NC_CACHE = build_nc()
    return _NC_CACHE


def kernel(x, mu, ro, mu_bias, ro_bias, eps, eps_bias, _trace=False, _tmpdir=None):
    x = np.ascontiguousarray(np.asarray(x, dtype=np.float32))
    mu = np.ascontiguousarray(np.asarray(mu, dtype=np.float32))
    ro = np.ascontiguousarray(np.asarray(ro, dtype=np.float32))
    mu_bias = np.ascontiguousarray(np.asarray(mu_bias, dtype=np.float32)).reshape(1, OUT)
    ro_bias = np.ascontiguousarray(np.asarray(ro_bias, dtype=np.float32)).reshape(1, OUT)
    eps = np.asarray(eps, dtype=np.float32)
    eps_bias = np.ascontiguousarray(np.asarray(eps_bias, dtype=np.float32))

    nc = _get_nc()

    in_maps = []
    for core in range(NCORES):
        b0, b1 = core * BS, (core + 1) * BS
        x_sh = x[b0:b1]  # (BS, IN)
        # xt[p, c*BS + b] = x_sh[b, c*128 + p]
        xt = np.ascontiguousarray(
            x_sh.reshape(BS, NCH, P).transpose(2, 1, 0).reshape(P, NCH * BS)
        )
        in_maps.append(
            {
                "eps": eps[b0:b1],
                "ro": ro,
                "mu": mu,
                "xt": xt,
                "eps_bias": eps_bias[b0:b1],
                "ro_bias": ro_bias,
                "mu_bias": mu_bias,
            }
        )

    res = run_bass_kernel_spmd(
        nc, in_maps, core_ids=list(range(NCORES)), trace=_trace, tmpdir=_tmpdir
    )
    out = np.concatenate([res.results[c]["out"] for c in range(NCORES)], axis=0)
    if _trace:
        kernel.last_results = res
    return out
